# revision 38
# baseline (speedup 1.0000x reference)
"""Trainium2 Bass kernel for nn_Chan_spaAtt (SE-gated conv block).

Key observation: the spatial self-attention branch in the reference is dead
code -- `gamma*attn_out + xo` is discarded and the output depends only on
xo = x * sigmoid(xl + xg) through the final 3x3 conv + BN + ReLU.

Computation per sample (C=64, H=W=64, N=4096), BN affines folded host-side:
  t1   = relu(W1 @ x + b1)            [16, N]
  sarg = W2 @ t1 + dbias              [64, N]
  dbias= G2 @ relu(G1 @ mean(x) + bg1) + bg2 + b2   [64, 1]
  xo   = x * sigmoid(sarg)            [64, N]
  y    = relu(conv3x3(xo, CW) + cb)   [64, N]

Layout: one sample per core (B=8). Row-pair interleaving puts EVEN image
rows on partitions 0:64 and ODD rows on partitions 64:128, so every
pointwise op uses all 128 partitions and halves its streamed columns.
The 3x3 conv runs as 6 K=128 x M=128 matmuls per 16-row chunk against a
padded buffer xop2 whose group g holds (top=xo[2g], bot=xo[2g-1]); the
SE-gate multiply writes xo straight into xop2 (partition-aligned), so no
SBUF->SBUF shuffle copies are needed. Everything flows in bf16 (rel err
~4e-3, gate 2e-2); psum accumulation stays f32.
"""

import sys

if "/opt/trn_rl_repo" not in sys.path:
    sys.path.insert(0, "/opt/trn_rl_repo")

import numpy as np
import ml_dtypes

import concourse.bass as bass
import concourse.bacc as bacc
import concourse.mybir as mybir
import concourse.tile as tile
from concourse.bass_utils import run_bass_kernel_spmd

B, C, H, W = 8, 64, 64, 64
N = H * W            # 4096
HALF = N // 2        # 2048 interleaved columns
INTER = 16
EPS = 1e-5
PW = W + 2           # 66 padded row stride
G = 33               # row-pair groups in xop2 (g: top=xo[2g], bot=xo[2g-1])
XCOLS = G * PW + 4   # 2182
NCHUNK = 4
CHUNK = HALF // NCHUNK   # 512
GPC = 8                  # groups per chunk

BF16 = mybir.dt.bfloat16
F32 = mybir.dt.float32
NPBF = ml_dtypes.bfloat16

# xaw (bf16): W1s rides in front of the first x half so one DMA sem gates mm1
O_W1S = 0                # [128, 32] block-diag W1^T
O_XA = 32                # x2 columns 0:1024
XAWCOLS = 32 + HALF // 2   # 1056
# wbr (bf16) column layout
O_W2S = 0                # [32, 128] block-diag W2^T (partitions 0:32)
O_CONV = 128             # 6 mats x [128,128]: E(-1),E(0),E(1),F(-1),F(0),F(1)
WBRCOLS = O_CONV + 6 * 128  # 896

# wf (f32) column layout
O_GW1S = 0               # [128, 16] stacked G1^T / N
O_GW2S = 16              # [33, 128]: rows 0:16 stacked G2^T, row 32 bsig
O_B1S = 144              # [32, 1]
O_CBS = 145              # [128, 1]
O_GB1S = 146             # [16, 1]
WFCOLS = 148

N_WARM = 6               # PE clock-warmup matmuls

_prog_cache = {}


def build_program(n_cores=8):
    nc = bacc.Bacc("TRN2", debug=False, target_bir_lowering=False,
                   num_devices=n_cores)

    wf_d = nc.dram_tensor("wf", [128, WFCOLS], F32, kind="ExternalInput").ap()
    xaw_d = nc.dram_tensor("xaw", [128, XAWCOLS], BF16, kind="ExternalInput").ap()
    x2b_d = nc.dram_tensor("x2b", [128, HALF // 2], BF16, kind="ExternalInput").ap()
    wbr_d = nc.dram_tensor("wbr", [128, WBRCOLS], BF16, kind="ExternalInput").ap()
    y2_d = nc.dram_tensor("y2", [128, HALF], BF16, kind="ExternalOutput").ap()

    with tile.TileContext(nc) as tc:
        with tc.tile_pool(name="big", bufs=1) as bpool, \
             tc.tile_pool(name="t1p", bufs=4) as tpool, \
             tc.tile_pool(name="sigp", bufs=3) as spool, \
             tc.tile_pool(name="yp", bufs=4) as ypool, \
             tc.tile_pool(name="ps1p", bufs=3, space="PSUM") as pp1, \
             tc.tile_pool(name="ps2p", bufs=2, space="PSUM") as pp2, \
             tc.tile_pool(name="psyp", bufs=2, space="PSUM") as ppy, \
             tc.tile_pool(name="psgp", bufs=1, space="PSUM") as ppg:

            wf = bpool.tile([128, WFCOLS], F32, tag="wf")
            xaw = bpool.tile([128, XAWCOLS], BF16, tag="xaw")
            x2b = bpool.tile([128, HALF // 2], BF16, tag="x2b")
            wbr = bpool.tile([128, WBRCOLS], BF16, tag="wbr")
            xop2 = bpool.tile([128, XCOLS], BF16, tag="xop2")
            g4 = bpool.tile([128, 3], F32, tag="g4")
            scr2 = bpool.tile([128, 512], BF16, tag="scr2")
            graw = bpool.tile([128, 1], F32, tag="graw")
            g1t = bpool.tile([33, 1], F32, tag="g1t")
            warm = bpool.tile([128, 512], BF16, tag="warm")
            scr = bpool.tile([1, 2], BF16, tag="scr")

            def xcol(c0, c1):
                """x2 column range [c0, c1) across the two landing tiles."""
                if c1 <= HALF // 2:
                    return xaw[:, O_XA + c0:O_XA + c1]
                assert c0 >= HALF // 2
                return x2b[:, c0 - HALF // 2:c1 - HALF // 2]

            # ---- input DMAs, all SP-issued: W1s+first-x-half ride one DMA
            # (a single sem gates mm1), second x half next (its sem gates the
            # global-mean chain), then remaining weights ----
            nc.sync.dma_start(xaw[:], xaw_d)
            nc.sync.dma_start(x2b[:], x2b_d)
            nc.sync.dma_start(wbr[:], wbr_d)
            nc.sync.dma_start(wf[:], wf_d)
            nc.gpsimd.memset(xop2[:], 0)

            # small zero-fills on DVE (keep Pool free for SWDGE gen)
            nc.vector.memset(warm[:], 0)
            nc.vector.memset(g1t[:], 0)
            nc.vector.memset(g1t[32:33, :], 1.0)

            # ---- activation-table preload: dummy sigmoid+relu force the
            # combined act-func-set load off the critical path ----
            nc.scalar.activation(scr[:, 0:1], warm[0:1, 0:1],
                                 mybir.ActivationFunctionType.Sigmoid)
            nc.scalar.activation(scr[:, 1:2], warm[0:1, 0:1],
                                 mybir.ActivationFunctionType.Relu)

            # ---- PE p-state warmup: idle PE runs at 0.65-1.2 GHz; sustained
            # busy reaches 2.4 GHz before the real matmuls ----
            warm_lhs = warm[:, 0:128]
            for i in range(N_WARM):
                pwarm = ppy.tile([128, 512], F32, tag="psy")
                nc.tensor.matmul(pwarm[:], warm_lhs, warm[:],
                                 start=True, stop=True)

            # ---- phase-1 mm1 for all chunks (PE order: before gmm) ----
            ps1s = {}
            for c in range(NCHUNK):
                ps1 = pp1.tile([32, CHUNK], F32, tag="ps1")
                nc.tensor.matmul(ps1[:], xaw[:, O_W1S:O_W1S + 32],
                                 xcol(c * CHUNK, (c + 1) * CHUNK),
                                 start=True, stop=True)
                ps1s[c] = ps1

            # ---- global-mean partial sums as x halves land: first half on
            # DVE; second half split Act (accum-copy) / DVE to halve the
            # post-arrival serial latency ----
            nc.vector.reduce_sum(g4[:, 0:1], xaw[:, O_XA:O_XA + 1024],
                                 axis=mybir.AxisListType.X)
            nc.scalar.activation(scr2[:], x2b[:, 0:512],
                                 mybir.ActivationFunctionType.Copy,
                                 accum_out=g4[:, 2:3])
            nc.vector.reduce_sum(g4[:, 1:2], x2b[:, 512:1024],
                                 axis=mybir.AxisListType.X)
            nc.vector.reduce_sum(graw[:], g4[:], axis=mybir.AxisListType.X)

            # ---- global branch: dbias = G2s@relu(G1s@sum(x)+gb1) + bsig ----
            pgt = ppg.tile([128, 2], F32, tag="pgt")
            nc.tensor.matmul(pgt[0:16, 0:1], wf[:, O_GW1S:O_GW1S + 16].bitcast(F32),
                             graw[:], start=True, stop=True)
            # g1relu on DVE (keeps Act free; fewer cross-engine hops)
            nc.vector.tensor_scalar(
                g1t[0:16, :], pgt[0:16, 0:1], wf[0:16, O_GB1S:O_GB1S + 1], 0.0,
                op0=mybir.AluOpType.add, op1=mybir.AluOpType.max)
            nc.tensor.matmul(pgt[:, 1:2], wf[0:33, O_GW2S:O_GW2S + 128].bitcast(F32),
                             g1t[:], start=True, stop=True, skip_group_check=True)
            dbias = bpool.tile([128, 1], F32, tag="dbias")
            nc.vector.tensor_scalar_add(dbias[:], pgt[:, 1:2], 0.0)

            # ---- t1 = relu(ps1 + b1): chunks 0-2 on the idle Pool engine,
            # chunk 3 on DVE right after dbias ----
            b1ap = wf[0:32, O_B1S:O_B1S + 1]
            t1s = {}
            for c in range(NCHUNK - 1):
                t1 = tpool.tile([32, CHUNK], BF16, tag="t1")
                nc.gpsimd.tensor_scalar(
                    t1[:], ps1s[c][:], b1ap, 0.0,
                    op0=mybir.AluOpType.add, op1=mybir.AluOpType.max)
                t1s[c] = t1
            t1 = tpool.tile([32, CHUNK], BF16, tag="t1")
            nc.vector.tensor_scalar(
                t1[:], ps1s[3][:], b1ap, 0.0,
                op0=mybir.AluOpType.add, op1=mybir.AluOpType.max)
            t1s[3] = t1

            ps2s = {}
            for c in range(NCHUNK):
                ps2 = pp2.tile([128, CHUNK], F32, tag="ps2")
                nc.tensor.matmul(ps2[:], wbr[0:32, O_W2S:O_W2S + 128],
                                 t1s.pop(c)[:], start=True, stop=True)
                ps2s[c] = ps2

            def rhs_slice(g0, dx, ngroups):
                base = g0 * PW + 1 + dx
                r = xop2[:, base:base + ngroups * PW]
                return r.rearrange("p (g w) -> p g w", w=PW)[:, :, 0:W]

            def conv_mm(psy_ap, mat, g0, dx, ngroups, start, stop, skip=False):
                nc.tensor.matmul(psy_ap,
                                 wbr[:, O_CONV + mat * 128:O_CONV + (mat + 1) * 128],
                                 rhs_slice(g0, dx, ngroups),
                                 start=start, stop=stop, skip_group_check=skip)

            def emit_conv_half(k, psy, h):
                # half h covers output pairs 4h..4h+3 (psum cols 256h..)
                ga = GPC * k + 4 * h
                lo, hi = 256 * h, 256 * h + 256
                for j, dx in enumerate((-1, 0, 1)):
                    conv_mm(psy[:, lo:hi], j, ga, dx, 4, j == 0, False)
                ng = 4 if h == 0 else 3
                for j, dx in enumerate((-1, 0, 1)):
                    conv_mm(psy[:, lo:hi - 64 * (h == 1)], 3 + j, ga + 1, dx,
                            ng, False, j == 2)

            def emit_conv_tail(k, psy, y_pool):
                for j, dx in enumerate((-1, 0, 1)):
                    conv_mm(psy[:, 448:512], 3 + j, GPC * k + GPC, dx, 1,
                            False, j == 2, skip=True)
                ybuf = y_pool.tile([128, 512], BF16, tag="ybuf")
                nc.scalar.activation(ybuf[:], psy[:],
                                     mybir.ActivationFunctionType.Relu,
                                     bias=wf[:, O_CBS:O_CBS + 1])
                nc.sync.dma_start(y2_d[:, k * 512:(k + 1) * 512], ybuf[:])

            # ---- sigmoid (Act) + gate-mul into xop2 (DVE) at 4-group
            # granularity; conv halves interleave so the PE starts as soon
            # as the first half-chunk of muls lands ----
            psys = {}
            for c in range(NCHUNK):
                sig = spool.tile([128, CHUNK], BF16, tag="sig")
                nc.scalar.activation(sig[:], ps2s.pop(c)[:],
                                     mybir.ActivationFunctionType.Sigmoid,
                                     bias=dbias[:])
                xr = xcol(c * CHUNK, (c + 1) * CHUNK)
                xr = xr.rearrange("p (g w) -> p g w", w=W)
                sr = sig[:].rearrange("p (g w) -> p g w", w=W)
                dt_ = xop2[0:C, GPC * c * PW + 1: GPC * c * PW + 1 + GPC * PW]
                dt_ = dt_.rearrange("p (g w) -> p g w", w=PW)[:, :, 0:W]
                db_ = xop2[C:2 * C, (GPC * c + 1) * PW + 1:
                           (GPC * c + 1) * PW + 1 + GPC * PW]
                db_ = db_.rearrange("p (g w) -> p g w", w=PW)[:, :, 0:W]

                # even g0 first: it is the previous chunk's F boundary group
                nc.vector.tensor_mul(dt_[:, 0:1], xr[0:C, 0:1], sr[0:C, 0:1])
                if c >= 1:
                    emit_conv_tail(c - 1, psys.pop(c - 1), ypool)
                nc.vector.tensor_mul(dt_[:, 1:4], xr[0:C, 1:4], sr[0:C, 1:4])
                nc.vector.tensor_mul(db_[:, 0:4], xr[C:2 * C, 0:4],
                                     sr[C:2 * C, 0:4])
                psy = ppy.tile([128, 512], F32, tag="psy")
                psys[c] = psy
                emit_conv_half(c, psy, 0)
                nc.vector.tensor_mul(dt_[:, 4:GPC], xr[0:C, 4:GPC],
                                     sr[0:C, 4:GPC])
                nc.vector.tensor_mul(db_[:, 4:GPC], xr[C:2 * C, 4:GPC],
                                     sr[C:2 * C, 4:GPC])
                emit_conv_half(c, psy, 1)
            emit_conv_tail(NCHUNK - 1, psys.pop(NCHUNK - 1), ypool)

    nc.compile()
    return nc


def _affine(s, b, m, v):
    inv = s / np.sqrt(v + EPS)
    return inv, b - m * inv


def prepare_weights(inputs):
    f = lambda k: np.asarray(inputs[k], dtype=np.float32)
    a1, c1 = _affine(f("ls1"), f("lbb1"), f("lm1"), f("lv1"))
    W1 = a1[:, None] * f("lw1")
    B1 = a1 * f("lb1") + c1
    a2, c2 = _affine(f("ls2"), f("lbb2"), f("lm2"), f("lv2"))
    W2 = a2[:, None] * f("lw2")
    B2 = a2 * f("lb2") + c2
    ag1, cg1 = _affine(f("gs1"), f("gbb1"), f("gm1"), f("gv1"))
    G1 = ag1[:, None] * f("gw1")
    Bg1 = ag1 * f("gb1") + cg1
    ag2, cg2 = _affine(f("gs2"), f("gbb2"), f("gm2"), f("gv2"))
    G2 = ag2[:, None] * f("gw2")
    Bg2 = ag2 * f("gb2") + cg2
    ac, cc = _affine(f("cs"), f("cbb"), f("cm"), f("cv"))
    CW = ac[:, None, None, None] * f("cw")        # [O, C, 3, 3]
    CB = ac * f("cb") + cc
    cwt = np.ascontiguousarray(
        CW.transpose(1, 2, 3, 0).reshape(C, 9 * C))  # [c, (ky kx) o]
    return {
        "w1t": np.ascontiguousarray(W1.T), "b1": B1,
        "w2t": np.ascontiguousarray(W2.T),
        "gw1t": np.ascontiguousarray(G1.T), "gb1": Bg1,
        "gw2t": np.ascontiguousarray(G2.T), "bsig": B2 + Bg2,
        "cwt": cwt, "cb": CB,
    }


def _blk(cwt, dy, dx):
    k9 = (dy + 1) * 3 + (dx + 1)
    return cwt[:, k9 * C:(k9 + 1) * C]


def assemble_w1s(sh):
    w1s = np.zeros((128, 32), np.float32)
    w1s[0:C, 0:INTER] = sh["w1t"]
    w1s[C:2 * C, INTER:32] = sh["w1t"]
    return w1s


def assemble_wbr(sh):
    wb = np.zeros((128, WBRCOLS), np.float32)
    wb[0:INTER, O_W2S:O_W2S + C] = sh["w2t"]
    wb[INTER:32, O_W2S + C:O_W2S + 2 * C] = sh["w2t"]
    cwt = sh["cwt"]
    for dx in (-1, 0, 1):
        e = np.zeros((128, 128), np.float32)
        e[0:C, 0:C] = _blk(cwt, 0, dx)
        e[0:C, C:2 * C] = _blk(cwt, -1, dx)
        e[C:2 * C, 0:C] = _blk(cwt, -1, dx)
        o = O_CONV + (dx + 1) * 128
        wb[:, o:o + 128] = e
        fmat = np.zeros((128, 128), np.float32)
        fmat[0:C, C:2 * C] = _blk(cwt, 1, dx)
        fmat[C:2 * C, 0:C] = _blk(cwt, 1, dx)
        fmat[C:2 * C, C:2 * C] = _blk(cwt, 0, dx)
        o = O_CONV + (3 + dx + 1) * 128
        wb[:, o:o + 128] = fmat
    return np.ascontiguousarray(wb.astype(NPBF))


def assemble_wf(sh):
    wf = np.zeros((128, WFCOLS), np.float32)
    g1n = sh["gw1t"] / N
    wf[0:C, O_GW1S:O_GW1S + INTER] = g1n
    wf[C:2 * C, O_GW1S:O_GW1S + INTER] = g1n
    wf[0:INTER, O_GW2S:O_GW2S + C] = sh["gw2t"]
    wf[0:INTER, O_GW2S + C:O_GW2S + 2 * C] = sh["gw2t"]
    wf[32, O_GW2S:O_GW2S + C] = sh["bsig"]
    wf[32, O_GW2S + C:O_GW2S + 2 * C] = sh["bsig"]
    wf[0:32, O_B1S] = np.concatenate([sh["b1"], sh["b1"]])
    wf[:, O_CBS] = np.concatenate([sh["cb"], sh["cb"]])
    wf[0:INTER, O_GB1S] = sh["gb1"]
    return np.ascontiguousarray(wf)


def interleave_x(xi):
    """[C, N] f32 -> [128, HALF] bf16: even rows on 0:64, odd on 64:128."""
    xr = xi.reshape(C, H // 2, 2, W)
    even = xr[:, :, 0, :].reshape(C, HALF)
    odd = xr[:, :, 1, :].reshape(C, HALF)
    return np.ascontiguousarray(
        np.concatenate([even, odd], axis=0).astype(NPBF))


def uninterleave_y(y2):
    """[128, HALF] -> [C, H, W] f32."""
    y = np.empty((C, H // 2, 2, W), np.float32)
    y[:, :, 0, :] = np.asarray(y2[0:C], np.float32).reshape(C, H // 2, W)
    y[:, :, 1, :] = np.asarray(y2[C:2 * C], np.float32).reshape(C, H // 2, W)
    return y.reshape(C, H, W)


def make_core_inputs(inputs):
    sh = prepare_weights(inputs)
    wbr = assemble_wbr(sh)
    wf = assemble_wf(sh)
    w1s = assemble_w1s(sh).astype(NPBF)
    x = np.asarray(inputs["x"], dtype=np.float32)
    maps = []
    for i in range(B):
        x2 = interleave_x(x[i].reshape(C, N))
        maps.append({
            "wf": wf, "wbr": wbr,
            "xaw": np.ascontiguousarray(
                np.concatenate([w1s, x2[:, 0:HALF // 2]], axis=1)),
            "x2b": np.ascontiguousarray(x2[:, HALF // 2:]),
        })
    return maps


def _run(inputs, trace=False):
    in_maps = make_core_inputs(inputs)
    if "prog" not in _prog_cache:
        _prog_cache["prog"] = build_program(B)
    nc = _prog_cache["prog"]
    res = run_bass_kernel_spmd(nc, in_maps, list(range(B)), trace=trace)
    out = np.stack([uninterleave_y(r["y2"]) for r in res.results])
    return out.astype(np.float32), res


def kernel(**inputs):
    out, _ = _run(inputs, trace=False)
    return out


def kernel_traced(inputs):
    return _run(inputs, trace=True)


def reference_numpy(inputs):
    """Pure-numpy emulation of the device dataflow (incl. bf16 rounding and
    the interleaved layouts) for fast algebra validation on host."""
    bf = lambda a: np.asarray(a, np.float32).astype(NPBF).astype(np.float32)
    in_maps = make_core_inputs(inputs)
    wf = in_maps[0]["wf"]
    wb = np.asarray(in_maps[0]["wbr"], np.float32)
    out = np.empty((B, C, H, W), np.float32)
    for i in range(B):
        x2 = np.concatenate(
            [np.asarray(in_maps[i]["xaw"], np.float32)[:, O_XA:],
             np.asarray(in_maps[i]["x2b"], np.float32)], axis=1)
        w1sf = np.asarray(in_maps[i]["xaw"], np.float32)[:, 0:32]
        graw = x2.sum(axis=1, keepdims=True)
        g1 = np.maximum(wf[:, O_GW1S:O_GW1S + 16].T @ graw
                        + wf[0:16, O_GB1S:O_GB1S + 1], 0.0)
        g1t = np.concatenate([g1, np.zeros((16, 1), np.float32),
                              np.ones((1, 1), np.float32)])
        dbias = wf[0:33, O_GW2S:O_GW2S + 128].T @ g1t      # [128, 1]
        ps1 = w1sf.T @ x2                                  # [32, HALF]
        t1 = bf(np.maximum(ps1 + wf[0:32, O_B1S:O_B1S + 1], 0.0))
        sarg = wb[0:32, O_W2S:O_W2S + 128].T @ t1 + dbias
        sig = bf(1.0 / (1.0 + np.exp(-sarg)))
        xop2 = np.zeros((128, XCOLS), np.float32)
        for c in range(NCHUNK):
            xo = bf(x2[:, c * CHUNK:(c + 1) * CHUNK]
                    * sig[:, c * CHUNK:(c + 1) * CHUNK])
            for g in range(GPC):
                ga = GPC * c + g
                xop2[0:C, ga * PW + 1:ga * PW + 1 + W] = xo[0:C, g * W:(g + 1) * W]
                gb = GPC * c + 1 + g
                xop2[C:2 * C, gb * PW + 1:gb * PW + 1 + W] = xo[C:2 * C, g * W:(g + 1) * W]
        y2 = np.empty((128, HALF), np.float32)
        for k in range(NCHUNK):
            psy = np.zeros((128, 512), np.float32)
            for j in range(6):
                ef, dx = divmod(j, 3)
                g0 = GPC * k + ef
                base = g0 * PW + 1 + (dx - 1)
                rhs = xop2[:, base:base + GPC * PW].reshape(128, GPC, PW)[:, :, 0:W]
                rhs = rhs.reshape(128, 512)
                psy += wb[:, O_CONV + j * 128:O_CONV + (j + 1) * 128].T @ rhs
            y2[:, k * 512:(k + 1) * 512] = np.maximum(
                psy + wf[:, O_CBS:O_CBS + 1], 0.0)
        out[i] = uninterleave_y(bf(y2))
    return out


# revision 39
# speedup vs baseline: 1.0812x; 1.0812x over previous
"""Trainium2 Bass kernel for nn_Chan_spaAtt (SE-gated conv block).

Key observation: the spatial self-attention branch in the reference is dead
code -- `gamma*attn_out + xo` is discarded and the output depends only on
xo = x * sigmoid(xl + xg) through the final 3x3 conv + BN + ReLU.

Computation per sample (C=64, H=W=64, N=4096), BN affines folded host-side:
  t1   = relu(W1 @ x + b1)            [16, N]
  sarg = W2 @ t1 + dbias              [64, N]
  dbias= G2 @ relu(G1 @ mean(x) + bg1) + bg2 + b2   [64, 1]
  xo   = x * sigmoid(sarg)            [64, N]
  y    = relu(conv3x3(xo, CW) + cb)   [64, N]

Layout: one sample per core (B=8). Row-pair interleaving puts EVEN image
rows on partitions 0:64 and ODD rows on partitions 64:128, so every
pointwise op uses all 128 partitions and halves its streamed columns.
The 3x3 conv runs as 6 K=128 x M=128 matmuls per 16-row chunk against a
padded buffer xop2 whose group g holds (top=xo[2g], bot=xo[2g-1]); the
SE-gate multiply writes xo straight into xop2 (partition-aligned), so no
SBUF->SBUF shuffle copies are needed. Everything flows in bf16 (rel err
~4e-3, gate 2e-2); psum accumulation stays f32.
"""

import sys

if "/opt/trn_rl_repo" not in sys.path:
    sys.path.insert(0, "/opt/trn_rl_repo")

import numpy as np
import ml_dtypes

import concourse.bass as bass
import concourse.bacc as bacc
import concourse.mybir as mybir
import concourse.tile as tile
from concourse.bass_utils import run_bass_kernel_spmd

B, C, H, W = 8, 64, 64, 64
N = H * W            # 4096
HALF = N // 2        # 2048 interleaved columns
INTER = 16
EPS = 1e-5
PW = W + 2           # 66 padded row stride
G = 33               # row-pair groups in xop2 (g: top=xo[2g], bot=xo[2g-1])
XCOLS = G * PW + 4   # 2182
NCHUNK = 4
CHUNK = HALF // NCHUNK   # 512
GPC = 8                  # groups per chunk

BF16 = mybir.dt.bfloat16
F32 = mybir.dt.float32
NPBF = ml_dtypes.bfloat16

# xaw (bf16): W1s rides in front of the first x half so one DMA sem gates mm1
O_W1S = 0                # [128, 32] block-diag W1^T
O_XA = 32                # x2 columns 0:1024
XAWCOLS = 32 + HALF // 2   # 1056
# wbr (bf16) column layout
O_W2S = 0                # [32, 128] block-diag W2^T (partitions 0:32)
O_CONV = 128             # 6 mats x [128,128]: E(-1),E(0),E(1),F(-1),F(0),F(1)
WBRCOLS = O_CONV + 6 * 128  # 896

# wf (f32) column layout
O_GW1S = 0               # [128, 16] stacked G1^T / N
O_GW2S = 16              # [33, 128]: rows 0:16 stacked G2^T, row 32 bsig
O_B1S = 144              # [32, 1]
O_CBS = 145              # [128, 1]
O_GB1S = 146             # [16, 1]
WFCOLS = 148

N_WARM = 6               # PE clock-warmup matmuls

_prog_cache = {}


def build_program(n_cores=8):
    nc = bacc.Bacc("TRN2", debug=False, target_bir_lowering=False,
                   num_devices=n_cores)

    wf_d = nc.dram_tensor("wf", [128, WFCOLS], F32, kind="ExternalInput").ap()
    xaw_d = nc.dram_tensor("xaw", [128, XAWCOLS], BF16, kind="ExternalInput").ap()
    x2b_d = nc.dram_tensor("x2b", [128, HALF // 2], BF16, kind="ExternalInput").ap()
    wbr_d = nc.dram_tensor("wbr", [128, WBRCOLS], BF16, kind="ExternalInput").ap()
    y2_d = nc.dram_tensor("y2", [128, HALF], BF16, kind="ExternalOutput").ap()

    with tile.TileContext(nc) as tc:
        with tc.tile_pool(name="big", bufs=1) as bpool, \
             tc.tile_pool(name="t1p", bufs=4) as tpool, \
             tc.tile_pool(name="sigp", bufs=3) as spool, \
             tc.tile_pool(name="yp", bufs=4) as ypool, \
             tc.tile_pool(name="ps1p", bufs=3, space="PSUM") as pp1, \
             tc.tile_pool(name="ps2p", bufs=2, space="PSUM") as pp2, \
             tc.tile_pool(name="psyp", bufs=2, space="PSUM") as ppy, \
             tc.tile_pool(name="psgp", bufs=1, space="PSUM") as ppg:

            wf = bpool.tile([128, WFCOLS], F32, tag="wf")
            xaw = bpool.tile([128, XAWCOLS], BF16, tag="xaw")
            x2b = bpool.tile([128, HALF // 2], BF16, tag="x2b")
            wbr = bpool.tile([128, WBRCOLS], BF16, tag="wbr")
            xop2 = bpool.tile([128, XCOLS], BF16, tag="xop2")
            g4 = bpool.tile([128, 3], F32, tag="g4")
            scr2 = bpool.tile([128, 512], BF16, tag="scr2")
            graw = bpool.tile([128, 1], F32, tag="graw")
            g1t = bpool.tile([33, 1], F32, tag="g1t")
            warm = bpool.tile([128, 512], BF16, tag="warm")
            scr = bpool.tile([1, 2], BF16, tag="scr")

            def xcol(c0, c1):
                """x2 column range [c0, c1) across the two landing tiles."""
                if c1 <= HALF // 2:
                    return xaw[:, O_XA + c0:O_XA + c1]
                assert c0 >= HALF // 2
                return x2b[:, c0 - HALF // 2:c1 - HALF // 2]

            # ---- input DMAs, all SP-issued: W1s+first-x-half ride one DMA
            # (a single sem gates mm1), second x half next (its sem gates the
            # global-mean chain), then remaining weights ----
            nc.sync.dma_start(xaw[:], xaw_d)
            nc.sync.dma_start(x2b[:], x2b_d)
            nc.sync.dma_start(wbr[:], wbr_d)
            nc.sync.dma_start(wf[:], wf_d)
            nc.gpsimd.memset(xop2[:], 0)

            # small zero-fills on DVE (keep Pool free for SWDGE gen)
            nc.vector.memset(warm[:], 0)
            nc.vector.memset(g1t[:], 0)
            nc.vector.memset(g1t[32:33, :], 1.0)

            # ---- activation-table preload: dummy sigmoid+relu force the
            # combined act-func-set load off the critical path ----
            nc.scalar.activation(scr[:, 0:1], warm[0:1, 0:1],
                                 mybir.ActivationFunctionType.Sigmoid)
            nc.scalar.activation(scr[:, 1:2], warm[0:1, 0:1],
                                 mybir.ActivationFunctionType.Relu)

            # ---- PE p-state warmup: idle PE runs at 0.65-1.2 GHz; sustained
            # busy reaches 2.4 GHz before the real matmuls ----
            warm_lhs = warm[:, 0:128]
            for i in range(N_WARM):
                pwarm = ppy.tile([128, 512], F32, tag="psy")
                nc.tensor.matmul(pwarm[:], warm_lhs, warm[:],
                                 start=True, stop=True)

            # ---- phase-1 mm1 for all chunks (PE order: before gmm) ----
            ps1s = {}
            for c in range(NCHUNK):
                ps1 = pp1.tile([32, CHUNK], F32, tag="ps1")
                nc.tensor.matmul(ps1[:], xaw[:, O_W1S:O_W1S + 32],
                                 xcol(c * CHUNK, (c + 1) * CHUNK),
                                 start=True, stop=True)
                ps1s[c] = ps1

            # ---- global-mean partial sums as x halves land: first half on
            # DVE; second half split Act (accum-copy) / DVE to halve the
            # post-arrival serial latency ----
            nc.vector.reduce_sum(g4[:, 0:1], xaw[:, O_XA:O_XA + 1024],
                                 axis=mybir.AxisListType.X)
            nc.scalar.activation(scr2[:], x2b[:, 0:512],
                                 mybir.ActivationFunctionType.Copy,
                                 accum_out=g4[:, 2:3])
            nc.vector.reduce_sum(g4[:, 1:2], x2b[:, 512:1024],
                                 axis=mybir.AxisListType.X)
            nc.vector.reduce_sum(graw[:], g4[:], axis=mybir.AxisListType.X)

            # ---- global branch: dbias = G2s@relu(G1s@sum(x)+gb1) + bsig ----
            pgt = ppg.tile([128, 2], F32, tag="pgt")
            nc.tensor.matmul(pgt[0:16, 0:1], wf[:, O_GW1S:O_GW1S + 16].bitcast(F32),
                             graw[:], start=True, stop=True)
            # g1relu on DVE (keeps Act free; fewer cross-engine hops)
            nc.vector.tensor_scalar(
                g1t[0:16, :], pgt[0:16, 0:1], wf[0:16, O_GB1S:O_GB1S + 1], 0.0,
                op0=mybir.AluOpType.add, op1=mybir.AluOpType.max)
            nc.tensor.matmul(pgt[:, 1:2], wf[0:33, O_GW2S:O_GW2S + 128].bitcast(F32),
                             g1t[:], start=True, stop=True, skip_group_check=True)
            dbias = bpool.tile([128, 1], F32, tag="dbias")
            nc.vector.tensor_scalar_add(dbias[:], pgt[:, 1:2], 0.0)

            # ---- t1 = relu(ps1 + b1): chunks 0-2 on the idle Pool engine,
            # chunk 3 on DVE right after dbias ----
            b1ap = wf[0:32, O_B1S:O_B1S + 1]
            t1s = {}
            for c in range(NCHUNK - 1):
                t1 = tpool.tile([32, CHUNK], BF16, tag="t1")
                nc.gpsimd.tensor_scalar(
                    t1[:], ps1s[c][:], b1ap, 0.0,
                    op0=mybir.AluOpType.add, op1=mybir.AluOpType.max)
                t1s[c] = t1
            t1 = tpool.tile([32, CHUNK], BF16, tag="t1")
            nc.vector.tensor_scalar(
                t1[:], ps1s[3][:], b1ap, 0.0,
                op0=mybir.AluOpType.add, op1=mybir.AluOpType.max)
            t1s[3] = t1

            ps2s = {}
            for c in range(NCHUNK):
                ps2 = pp2.tile([128, CHUNK], F32, tag="ps2")
                nc.tensor.matmul(ps2[:], wbr[0:32, O_W2S:O_W2S + 128],
                                 t1s.pop(c)[:], start=True, stop=True)
                ps2s[c] = ps2

            def rhs_slice(g0, dx, ngroups):
                base = g0 * PW + 1 + dx
                r = xop2[:, base:base + ngroups * PW]
                return r.rearrange("p (g w) -> p g w", w=PW)[:, :, 0:W]

            def conv_mm(psy_ap, mat, g0, dx, ngroups, start, stop, skip=False):
                nc.tensor.matmul(psy_ap,
                                 wbr[:, O_CONV + mat * 128:O_CONV + (mat + 1) * 128],
                                 rhs_slice(g0, dx, ngroups),
                                 start=start, stop=stop, skip_group_check=skip)

            def emit_conv_half(k, psy, h):
                # half h: E covers output pairs 4h..4h+3 (psum cols 256h..);
                # F lags one group (its group g feeds pair g-8k-1, and group
                # 8k+4 is written by the h=1 muls), so h=0 does 3 F groups
                # ([0:192]) and h=1 does 4 ([192:448], crossing the E region
                # boundary -> skip the sim's psum group check).
                ga = GPC * k + 4 * h
                lo = 256 * h
                for j, dx in enumerate((-1, 0, 1)):
                    conv_mm(psy[:, lo:lo + 256], j, ga, dx, 4, j == 0, False)
                if h == 0:
                    for j, dx in enumerate((-1, 0, 1)):
                        conv_mm(psy[:, 0:192], 3 + j, ga + 1, dx, 3,
                                False, j == 2)
                else:
                    for j, dx in enumerate((-1, 0, 1)):
                        conv_mm(psy[:, 192:448], 3 + j, ga, dx, 4,
                                False, j == 2, skip=True)

            def emit_conv_tail(k, psy, y_pool):
                for j, dx in enumerate((-1, 0, 1)):
                    conv_mm(psy[:, 448:512], 3 + j, GPC * k + GPC, dx, 1,
                            False, j == 2, skip=True)
                ybuf = y_pool.tile([128, 512], BF16, tag="ybuf")
                nc.scalar.activation(ybuf[:], psy[:],
                                     mybir.ActivationFunctionType.Relu,
                                     bias=wf[:, O_CBS:O_CBS + 1])
                nc.sync.dma_start(y2_d[:, k * 512:(k + 1) * 512], ybuf[:])

            # ---- sigmoid (Act) + gate-mul into xop2 (DVE) at 4-group
            # granularity; conv halves interleave so the PE starts as soon
            # as the first half-chunk of muls lands ----
            psys = {}
            for c in range(NCHUNK):
                sig = spool.tile([128, CHUNK], BF16, tag="sig")
                nc.scalar.activation(sig[:], ps2s.pop(c)[:],
                                     mybir.ActivationFunctionType.Sigmoid,
                                     bias=dbias[:])
                xr = xcol(c * CHUNK, (c + 1) * CHUNK)
                xr = xr.rearrange("p (g w) -> p g w", w=W)
                sr = sig[:].rearrange("p (g w) -> p g w", w=W)
                dt_ = xop2[0:C, GPC * c * PW + 1: GPC * c * PW + 1 + GPC * PW]
                dt_ = dt_.rearrange("p (g w) -> p g w", w=PW)[:, :, 0:W]
                db_ = xop2[C:2 * C, (GPC * c + 1) * PW + 1:
                           (GPC * c + 1) * PW + 1 + GPC * PW]
                db_ = db_.rearrange("p (g w) -> p g w", w=PW)[:, :, 0:W]

                # even g0 first: it is the previous chunk's F boundary group
                nc.vector.tensor_mul(dt_[:, 0:1], xr[0:C, 0:1], sr[0:C, 0:1])
                if c >= 1:
                    emit_conv_tail(c - 1, psys.pop(c - 1), ypool)
                nc.vector.tensor_mul(dt_[:, 1:4], xr[0:C, 1:4], sr[0:C, 1:4])
                nc.vector.tensor_mul(db_[:, 0:4], xr[C:2 * C, 0:4],
                                     sr[C:2 * C, 0:4])
                psy = ppy.tile([128, 512], F32, tag="psy")
                psys[c] = psy
                emit_conv_half(c, psy, 0)
                nc.vector.tensor_mul(dt_[:, 4:GPC], xr[0:C, 4:GPC],
                                     sr[0:C, 4:GPC])
                nc.vector.tensor_mul(db_[:, 4:GPC], xr[C:2 * C, 4:GPC],
                                     sr[C:2 * C, 4:GPC])
                emit_conv_half(c, psy, 1)
            emit_conv_tail(NCHUNK - 1, psys.pop(NCHUNK - 1), ypool)

    nc.compile()
    return nc


def _affine(s, b, m, v):
    inv = s / np.sqrt(v + EPS)
    return inv, b - m * inv


def prepare_weights(inputs):
    f = lambda k: np.asarray(inputs[k], dtype=np.float32)
    a1, c1 = _affine(f("ls1"), f("lbb1"), f("lm1"), f("lv1"))
    W1 = a1[:, None] * f("lw1")
    B1 = a1 * f("lb1") + c1
    a2, c2 = _affine(f("ls2"), f("lbb2"), f("lm2"), f("lv2"))
    W2 = a2[:, None] * f("lw2")
    B2 = a2 * f("lb2") + c2
    ag1, cg1 = _affine(f("gs1"), f("gbb1"), f("gm1"), f("gv1"))
    G1 = ag1[:, None] * f("gw1")
    Bg1 = ag1 * f("gb1") + cg1
    ag2, cg2 = _affine(f("gs2"), f("gbb2"), f("gm2"), f("gv2"))
    G2 = ag2[:, None] * f("gw2")
    Bg2 = ag2 * f("gb2") + cg2
    ac, cc = _affine(f("cs"), f("cbb"), f("cm"), f("cv"))
    CW = ac[:, None, None, None] * f("cw")        # [O, C, 3, 3]
    CB = ac * f("cb") + cc
    cwt = np.ascontiguousarray(
        CW.transpose(1, 2, 3, 0).reshape(C, 9 * C))  # [c, (ky kx) o]
    return {
        "w1t": np.ascontiguousarray(W1.T), "b1": B1,
        "w2t": np.ascontiguousarray(W2.T),
        "gw1t": np.ascontiguousarray(G1.T), "gb1": Bg1,
        "gw2t": np.ascontiguousarray(G2.T), "bsig": B2 + Bg2,
        "cwt": cwt, "cb": CB,
    }


def _blk(cwt, dy, dx):
    k9 = (dy + 1) * 3 + (dx + 1)
    return cwt[:, k9 * C:(k9 + 1) * C]


def assemble_w1s(sh):
    w1s = np.zeros((128, 32), np.float32)
    w1s[0:C, 0:INTER] = sh["w1t"]
    w1s[C:2 * C, INTER:32] = sh["w1t"]
    return w1s


def assemble_wbr(sh):
    wb = np.zeros((128, WBRCOLS), np.float32)
    wb[0:INTER, O_W2S:O_W2S + C] = sh["w2t"]
    wb[INTER:32, O_W2S + C:O_W2S + 2 * C] = sh["w2t"]
    cwt = sh["cwt"]
    for dx in (-1, 0, 1):
        e = np.zeros((128, 128), np.float32)
        e[0:C, 0:C] = _blk(cwt, 0, dx)
        e[0:C, C:2 * C] = _blk(cwt, -1, dx)
        e[C:2 * C, 0:C] = _blk(cwt, -1, dx)
        o = O_CONV + (dx + 1) * 128
        wb[:, o:o + 128] = e
        fmat = np.zeros((128, 128), np.float32)
        fmat[0:C, C:2 * C] = _blk(cwt, 1, dx)
        fmat[C:2 * C, 0:C] = _blk(cwt, 1, dx)
        fmat[C:2 * C, C:2 * C] = _blk(cwt, 0, dx)
        o = O_CONV + (3 + dx + 1) * 128
        wb[:, o:o + 128] = fmat
    return np.ascontiguousarray(wb.astype(NPBF))


def assemble_wf(sh):
    wf = np.zeros((128, WFCOLS), np.float32)
    g1n = sh["gw1t"] / N
    wf[0:C, O_GW1S:O_GW1S + INTER] = g1n
    wf[C:2 * C, O_GW1S:O_GW1S + INTER] = g1n
    wf[0:INTER, O_GW2S:O_GW2S + C] = sh["gw2t"]
    wf[0:INTER, O_GW2S + C:O_GW2S + 2 * C] = sh["gw2t"]
    wf[32, O_GW2S:O_GW2S + C] = sh["bsig"]
    wf[32, O_GW2S + C:O_GW2S + 2 * C] = sh["bsig"]
    wf[0:32, O_B1S] = np.concatenate([sh["b1"], sh["b1"]])
    wf[:, O_CBS] = np.concatenate([sh["cb"], sh["cb"]])
    wf[0:INTER, O_GB1S] = sh["gb1"]
    return np.ascontiguousarray(wf)


def interleave_x(xi):
    """[C, N] f32 -> [128, HALF] bf16: even rows on 0:64, odd on 64:128."""
    xr = xi.reshape(C, H // 2, 2, W)
    even = xr[:, :, 0, :].reshape(C, HALF)
    odd = xr[:, :, 1, :].reshape(C, HALF)
    return np.ascontiguousarray(
        np.concatenate([even, odd], axis=0).astype(NPBF))


def uninterleave_y(y2):
    """[128, HALF] -> [C, H, W] f32."""
    y = np.empty((C, H // 2, 2, W), np.float32)
    y[:, :, 0, :] = np.asarray(y2[0:C], np.float32).reshape(C, H // 2, W)
    y[:, :, 1, :] = np.asarray(y2[C:2 * C], np.float32).reshape(C, H // 2, W)
    return y.reshape(C, H, W)


def make_core_inputs(inputs):
    sh = prepare_weights(inputs)
    wbr = assemble_wbr(sh)
    wf = assemble_wf(sh)
    w1s = assemble_w1s(sh).astype(NPBF)
    x = np.asarray(inputs["x"], dtype=np.float32)
    maps = []
    for i in range(B):
        x2 = interleave_x(x[i].reshape(C, N))
        maps.append({
            "wf": wf, "wbr": wbr,
            "xaw": np.ascontiguousarray(
                np.concatenate([w1s, x2[:, 0:HALF // 2]], axis=1)),
            "x2b": np.ascontiguousarray(x2[:, HALF // 2:]),
        })
    return maps


def _run(inputs, trace=False):
    in_maps = make_core_inputs(inputs)
    if "prog" not in _prog_cache:
        _prog_cache["prog"] = build_program(B)
    nc = _prog_cache["prog"]
    res = run_bass_kernel_spmd(nc, in_maps, list(range(B)), trace=trace)
    out = np.stack([uninterleave_y(r["y2"]) for r in res.results])
    return out.astype(np.float32), res


def kernel(**inputs):
    out, _ = _run(inputs, trace=False)
    return out


def kernel_traced(inputs):
    return _run(inputs, trace=True)


def reference_numpy(inputs):
    """Pure-numpy emulation of the device dataflow (incl. bf16 rounding and
    the interleaved layouts) for fast algebra validation on host."""
    bf = lambda a: np.asarray(a, np.float32).astype(NPBF).astype(np.float32)
    in_maps = make_core_inputs(inputs)
    wf = in_maps[0]["wf"]
    wb = np.asarray(in_maps[0]["wbr"], np.float32)
    out = np.empty((B, C, H, W), np.float32)
    for i in range(B):
        x2 = np.concatenate(
            [np.asarray(in_maps[i]["xaw"], np.float32)[:, O_XA:],
             np.asarray(in_maps[i]["x2b"], np.float32)], axis=1)
        w1sf = np.asarray(in_maps[i]["xaw"], np.float32)[:, 0:32]
        graw = x2.sum(axis=1, keepdims=True)
        g1 = np.maximum(wf[:, O_GW1S:O_GW1S + 16].T @ graw
                        + wf[0:16, O_GB1S:O_GB1S + 1], 0.0)
        g1t = np.concatenate([g1, np.zeros((16, 1), np.float32),
                              np.ones((1, 1), np.float32)])
        dbias = wf[0:33, O_GW2S:O_GW2S + 128].T @ g1t      # [128, 1]
        ps1 = w1sf.T @ x2                                  # [32, HALF]
        t1 = bf(np.maximum(ps1 + wf[0:32, O_B1S:O_B1S + 1], 0.0))
        sarg = wb[0:32, O_W2S:O_W2S + 128].T @ t1 + dbias
        sig = bf(1.0 / (1.0 + np.exp(-sarg)))
        xop2 = np.zeros((128, XCOLS), np.float32)
        for c in range(NCHUNK):
            xo = bf(x2[:, c * CHUNK:(c + 1) * CHUNK]
                    * sig[:, c * CHUNK:(c + 1) * CHUNK])
            for g in range(GPC):
                ga = GPC * c + g
                xop2[0:C, ga * PW + 1:ga * PW + 1 + W] = xo[0:C, g * W:(g + 1) * W]
                gb = GPC * c + 1 + g
                xop2[C:2 * C, gb * PW + 1:gb * PW + 1 + W] = xo[C:2 * C, g * W:(g + 1) * W]
        y2 = np.empty((128, HALF), np.float32)
        for k in range(NCHUNK):
            psy = np.zeros((128, 512), np.float32)
            for j in range(6):
                ef, dx = divmod(j, 3)
                g0 = GPC * k + ef
                base = g0 * PW + 1 + (dx - 1)
                rhs = xop2[:, base:base + GPC * PW].reshape(128, GPC, PW)[:, :, 0:W]
                rhs = rhs.reshape(128, 512)
                psy += wb[:, O_CONV + j * 128:O_CONV + (j + 1) * 128].T @ rhs
            y2[:, k * 512:(k + 1) * 512] = np.maximum(
                psy + wf[:, O_CBS:O_CBS + 1], 0.0)
        out[i] = uninterleave_y(bf(y2))
    return out
